# revision 23
# baseline (speedup 1.0000x reference)
"""Causal GQA attention (S=2048, B=2, HQ=32, HKV=8, D=128) on 8 trn2 cores.

Sharding: the 16 (batch, kv-head) pairs are split 2 per core (data+head
parallel). Each pair carries group=4 query heads -> 8 attention heads/core.

Device kernel per head computes S^T = (Q K^T)^T chunk-group by chunk-group
in PSUM (so the softmax free axis never needs an on-chip transpose),
exponentiates on ACT into SBUF (P^T, fp16), and accumulates
out^T = V^T P^T matmuls with V stationary, one group behind the QK stream
so the PE never waits on ACT.  The causal mask is applied INSIDE the QK
accumulation: one extra matmul per diagonal block adds -BIG onto the
strict-lower triangle of S^T (lhsT=I, rhs=trim), so exp produces exact
zeros there and nothing sits between exp and the AV matmuls.  The softmax
denominators are NOT computed with PE ones-matmuls: the Vector engine
accumulates R = sum_ktile P^T (elementwise fp16 adds, 2-byte DVE fast
path), R is DMA'd out, and the host finishes denom = R.sum(partitions) and
the divide.  All matmul operands are fp16 (1 col/cycle on the PE; more
precise than bf16 at these magnitudes), PSUM accumulation fp32; Q/K/V are
pre-cast to fp16 on the host, halving input DMA bytes.

PSUM budget/partition: 2 x [128,1024] f32 S^T staging (8KB) + 4 x [128,512]
f32 out accumulators (8KB) = 16KB exactly.

Host side only re-lays-out data: Q/K are fed pre-transposed [d, s], V as
[k_local, ktile, d]; the returned out^T [d, s] (unnormalized, fp16) is
divided by the denominators and transposed back.
"""

import numpy as np

import concourse.bass as bass
import concourse.mybir as mybir
import concourse.tile as tile
from concourse import bacc, bass_utils
from concourse.masks import make_identity, make_lower_triangular

S, B, HQ, HKV, D = 2048, 2, 32, 8, 128
G = HQ // HKV                      # 4 query heads per kv head
NCORES = 8
NPAIRS = B * HKV                   # 16 (batch, kv-head) pairs
PAIRS_PER_CORE = NPAIRS // NCORES  # 2
HEADS_PER_CORE = PAIRS_PER_CORE * G  # 8
SCALE = 1.0 / float(np.sqrt(D))
QC = 512                           # q-chunk (PSUM bank) width
NQC = S // QC                      # 4
KT = 128                           # k-tile (partition) width
NKT = S // KT                      # 16

F32 = mybir.dt.float32
F16 = mybir.dt.float16


def head_groups():
    """Chunk groups per head: (kti, chunks, off).

    For k-tile kti (k rows [w, w+128)), live q columns are [w, S).  Chunks of
    512 q-columns c0..3 (c0 = kti//4) are processed in pairs so consecutive
    matmuls share the stationary operand and exp covers 1024 columns.  off is
    the causal offset inside the first chunk of the FIRST group only.
    """
    groups = []
    for kti in range(NKT):
        w = KT * kti
        c0 = w // QC
        off = w - QC * c0
        cs = list(range(c0, NQC))
        first = True
        while cs:
            take = cs[:2]
            cs = cs[2:]
            groups.append((kti, tuple(take), off if first else 0))
            first = False
    return groups


GROUPS = head_groups()  # 24 groups


def emit_core_program(tc, qt, kt, v, ot, rd):
    """qt: [8, D, S] f16 DRAM (Q^T per head), kt: [2, D, S] f16, v: [2, 128,
    NKT*D] f16, ot: [8, D, S] f16 out (unnormalized out^T), rd: [8, 128, S]
    f16 out (per-k-row partial exp sums; host reduces partitions)."""
    from contextlib import ExitStack

    nc = tc.nc
    with ExitStack() as ctx:
        _emit(ctx, tc, nc, qt, kt, v, ot, rd)


def _emit(ctx, tc, nc, qt, kt, v, ot, rd):
    singles = ctx.enter_context(tc.tile_pool(name="singles", bufs=1))
    kv_pool = ctx.enter_context(tc.tile_pool(name="kv", bufs=2))
    q_pool = ctx.enter_context(tc.tile_pool(name="q", bufs=3))
    p_pool = ctx.enter_context(tc.tile_pool(name="p", bufs=8))
    r_pool = ctx.enter_context(tc.tile_pool(name="r", bufs=2))
    ob_pool = ctx.enter_context(tc.tile_pool(name="ob", bufs=4))
    ps_s = ctx.enter_context(tc.tile_pool(name="ps_s", bufs=2, space="PSUM"))
    ps_o = ctx.enter_context(tc.tile_pool(name="ps_o", bufs=4, space="PSUM"))

    # One-group-deep software pipeline over the whole program: the AV
    # matmuls for group i are emitted after the QK matmuls of group i+1, so
    # the PE always has runnable work while ACT exponentiates group i+1.
    pending = []  # [(head_ctx, kti, cs, off, p_tile)]

    def flush_pending():
        hctx, kti, cs, off, p_t = pending.pop(0)
        v_sb_, o_tiles_, head_ = hctx["v_sb"], hctx["o"], hctx["head"]
        for j, c in enumerate(cs):
            o_c = off if j == 0 else 0
            t0 = QC * j + o_c
            nc.tensor.matmul(
                out=o_tiles_[c][:, o_c:QC],
                lhsT=v_sb_[:, D * kti:D * (kti + 1)],
                rhs=p_t[:, t0:QC * (j + 1)],
                start=(kti == 0), stop=(kti == 4 * c + 3),
            )
            if kti == 4 * c + 3:
                # chunk finished accumulating: drain PSUM -> SBUF f16 -> DRAM
                osb = ob_pool.tile([128, QC], F16, tag="osb",
                                   name=f"osb_{head_}_{c}")
                nc.vector.tensor_copy(out=osb[:], in_=o_tiles_[c][:])
                nc.sync.dma_start(
                    out=ot[head_][:, QC * c:QC * (c + 1)], in_=osb[:])

    # Input loads (fp16, pre-cast on host) are prefetched one head ahead.
    pair_res = {}
    q_res = {}

    def load_pair(pr, splits=(S,)):
        kt_sb = kv_pool.tile([D, S], F16, tag="kt", name=f"kt_{pr}")
        lo = 0
        for hi in splits:
            nc.gpsimd.dma_start(out=kt_sb[:, lo:hi], in_=kt[pr][:, lo:hi])
            lo = hi
        v_sb = kv_pool.tile([128, NKT * D], F16, tag="v", name=f"v_{pr}")
        nc.gpsimd.dma_start(out=v_sb[:], in_=v[pr])
        pair_res[pr] = (kt_sb, v_sb)

    def load_q(h, splits=(S,)):
        q_sb = q_pool.tile([D, S], F16, tag="q", name=f"q_{h}")
        lo = 0
        for hi in splits:
            nc.sync.dma_start(out=q_sb[:, lo:hi], in_=qt[h][:, lo:hi])
            lo = hi
        q_res[h] = q_sb

    # Startup: issue the first loads before anything else queues on the
    # DMA-trigger engines, split into column chunks so the first QK group
    # only waits for kt cols [0,128) + q cols [0,512).
    load_pair(0, splits=(KT, QC, 2 * QC, S))
    load_q(0, splits=(QC, 2 * QC, S))

    # Causal mask is applied INSIDE the QK accumulation: one extra matmul
    # adds trim[k, q] = -BIG where q < k (strict lower triangle) to the
    # S^T diagonal block in PSUM, so exp produces exact zeros there and
    # nothing sits between exp and the AV matmuls.
    identf = singles.tile([128, 128], F32)
    make_identity(nc, identf[:])
    ident = singles.tile([128, 128], F16)
    nc.scalar.copy(out=ident[:], in_=identf[:])
    trimf = singles.tile([128, 128], F32)
    make_lower_triangular(nc, trimf[:], val=-30000.0, diag=False)
    trim = singles.tile([128, 128], F16)
    nc.scalar.copy(out=trim[:], in_=trimf[:])
    for head in range(HEADS_PER_CORE):
        pair = head // G
        if head + 1 < HEADS_PER_CORE:
            if (head + 1) // G != pair:
                load_pair((head + 1) // G)
            load_q(head + 1)
        if True:
            kt_sb, v_sb = pair_res[pair]
            q_sb = q_res.pop(head)
            r_sb = r_pool.tile([128, S], F16, tag="r", name=f"r_{head}")
            o_tiles = [ps_o.tile([128, QC], F32, tag="o", name=f"o_{head}_{c}")
                       for c in range(NQC)]
            hctx = {"v_sb": v_sb, "o": o_tiles, "head": head}

            for kti, cs, off in GROUPS:
                w = KT * kti
                ncols = QC * len(cs)
                s_t = ps_s.tile([128, 2 * QC], F32, tag="s",
                                name=f"s_{head}_{kti}_{cs[0]}")
                # QK^T: consecutive matmuls share lhsT (k-tile of K^T).
                # The -BIG causal-mask matmul OPENS the diagonal block's
                # accumulation chain (start=True) and the QK matmul over the
                # same exact region closes it, so the mask never sits last
                # before exp — exp's dependency is the final QK matmul.
                # The chain members stay contiguous on the PE (interleaving
                # another matmul inside an open group faults the exec unit).
                has_diag = QC * cs[0] <= w < QC * (cs[0] + 1)
                if has_diag:
                    nc.tensor.matmul(
                        out=s_t[:, off:off + KT],
                        lhsT=ident[:],
                        rhs=trim[:],
                        start=True, stop=False,
                    )
                    nc.tensor.matmul(
                        out=s_t[:, off:off + KT],
                        lhsT=kt_sb[:, w:w + KT],
                        rhs=q_sb[:, QC * cs[0] + off:QC * cs[0] + off + KT],
                        start=False, stop=True,
                    )
                    if off + KT < QC:
                        nc.tensor.matmul(
                            out=s_t[:, off + KT:QC],
                            lhsT=kt_sb[:, w:w + KT],
                            rhs=q_sb[:, QC * cs[0] + off + KT:QC * (cs[0] + 1)],
                            start=True, stop=True,
                        )
                    rest = list(range(1, len(cs)))
                else:
                    rest = list(range(len(cs)))
                for j in rest:
                    c = cs[j]
                    o_c = off if j == 0 else 0
                    nc.tensor.matmul(
                        out=s_t[:, QC * j + o_c:QC * (j + 1)],
                        lhsT=kt_sb[:, w:w + KT],
                        rhs=q_sb[:, QC * c + o_c:QC * (c + 1)],
                        start=True, stop=True,
                    )
                # exp on ACT (one instruction for the whole group)
                p_t = p_pool.tile([128, 2 * QC], F16, tag="p",
                                  name=f"p_{head}_{kti}_{cs[0]}")
                nc.scalar.activation(
                    p_t[:, off:ncols], s_t[:, off:ncols],
                    mybir.ActivationFunctionType.Exp, scale=SCALE)
                # denominator partials: R += P^T (Vector engine, fp16)
                rcol = slice(QC * cs[0] + off, QC * (cs[-1] + 1))
                if kti == 0:
                    nc.vector.tensor_copy(
                        out=r_sb[:, rcol], in_=p_t[:, off:ncols])
                else:
                    nc.vector.tensor_add(
                        r_sb[:, rcol], r_sb[:, rcol], p_t[:, off:ncols])

                pending.append((hctx, kti, cs, off, p_t))
                if len(pending) > 1:
                    flush_pending()

            # R is complete once the last group's add ran; DMA it out
            nc.sync.dma_start(out=rd[head], in_=r_sb[:])


    while pending:
        flush_pending()


_CACHED_NC = None


def build_program():
    global _CACHED_NC
    if _CACHED_NC is not None:
        return _CACHED_NC
    nc = bacc.Bacc("TRN2", target_bir_lowering=False, debug=False,
                   num_devices=NCORES)
    qt = nc.dram_tensor("qt", [HEADS_PER_CORE, D, S], F16,
                        kind="ExternalInput").ap()
    kt = nc.dram_tensor("kt", [PAIRS_PER_CORE, D, S], F16,
                        kind="ExternalInput").ap()
    v = nc.dram_tensor("v", [PAIRS_PER_CORE, 128, NKT * D], F16,
                       kind="ExternalInput").ap()
    ot = nc.dram_tensor("ot", [HEADS_PER_CORE, D, S], F16,
                        kind="ExternalOutput").ap()
    rd = nc.dram_tensor("rd", [HEADS_PER_CORE, 128, S], F16,
                        kind="ExternalOutput").ap()
    with tile.TileContext(nc) as tc:
        emit_core_program(tc, qt, kt, v, ot, rd)
    nc.compile()
    _CACHED_NC = nc
    return nc


def shard_inputs(query, key, value):
    """Full inputs -> list of 8 per-core in_maps (host-side relayout only)."""
    query = np.asarray(query, dtype=np.float32)
    key = np.asarray(key, dtype=np.float32)
    value = np.asarray(value, dtype=np.float32)

    # Q: [S,B,HQ,D] -> [B*HKV, G, D, S]
    qtall = np.ascontiguousarray(
        query.reshape(S, B, HKV, G, D).transpose(1, 2, 3, 4, 0),
        dtype=np.float16).reshape(NPAIRS, G, D, S)
    # K: [S,B,HKV,D] -> [B*HKV, D, S]
    ktall = np.ascontiguousarray(
        key.transpose(1, 2, 3, 0), dtype=np.float16).reshape(NPAIRS, D, S)
    # V: [S,B,HKV,D] -> [B*HKV, k_local=128, NKT*D]
    vall = np.ascontiguousarray(
        value.reshape(NKT, 128, B, HKV, D).transpose(2, 3, 1, 0, 4),
        dtype=np.float16).reshape(NPAIRS, 128, NKT * D)

    in_maps = []
    for c in range(NCORES):
        p0 = PAIRS_PER_CORE * c
        p1 = p0 + PAIRS_PER_CORE
        in_maps.append({
            "qt": np.ascontiguousarray(qtall[p0:p1].reshape(HEADS_PER_CORE, D, S)),
            "kt": np.ascontiguousarray(ktall[p0:p1]),
            "v": np.ascontiguousarray(vall[p0:p1]),
        })
    return in_maps


def unshard_output(results):
    """8 per-core {'ot','rd'} -> full [S, B, HQ, D] (normalize on host)."""
    ot = np.stack([np.asarray(r["ot"], dtype=np.float32) for r in results])
    rd = np.stack([np.asarray(r["rd"], dtype=np.float32) for r in results])
    denom = rd.sum(axis=2)                         # [8, 8, S]
    ot /= denom[:, :, None, :]                     # [8, 8, D, S]
    ot = ot.reshape(B, HKV, G, D, S)               # pairs major -> b, hkv
    out = np.ascontiguousarray(ot.transpose(4, 0, 1, 2, 3))  # [S,B,HKV,G,D]
    return out.reshape(S, B, HQ, D)


def kernel(query, key, value, _trace=False, _return_bkr=False):
    nc = build_program()
    in_maps = shard_inputs(query, key, value)
    bkr = bass_utils.run_bass_kernel_spmd(
        nc, in_maps, core_ids=list(range(NCORES)), trace=_trace)
    out = unshard_output(bkr.results)
    if _return_bkr:
        return out, bkr
    return out


if __name__ == "__main__":
    q = np.random.randn(S, B, HQ, D).astype(np.float32)
    k = np.random.randn(S, B, HKV, D).astype(np.float32)
    vv = np.random.randn(S, B, HKV, D).astype(np.float32)
    o = kernel(q, k, vv)
    print("out", o.shape, o.dtype, float(np.abs(o).max()))


# revision 24
# speedup vs baseline: 1.2009x; 1.2009x over previous
"""Causal GQA attention (S=2048, B=2, HQ=32, HKV=8, D=128) on 8 trn2 cores.

Sharding: the 16 (batch, kv-head) pairs are split 2 per core (data+head
parallel). Each pair carries group=4 query heads -> 8 attention heads/core.

Device kernel per head computes S^T = (Q K^T)^T chunk-group by chunk-group
in PSUM (so the softmax free axis never needs an on-chip transpose),
exponentiates on ACT into SBUF (P^T, fp16), and accumulates
out^T = V^T P^T matmuls with V stationary, one group behind the QK stream
so the PE never waits on ACT.  The causal mask is applied INSIDE the QK
accumulation: one extra matmul per diagonal block adds -BIG onto the
strict-lower triangle of S^T (lhsT=I, rhs=trim), so exp produces exact
zeros there and nothing sits between exp and the AV matmuls.  The softmax
denominators are NOT computed with PE ones-matmuls: the Vector engine
accumulates R = sum_ktile P^T (elementwise fp16 adds, 2-byte DVE fast
path), R is DMA'd out, and the host finishes denom = R.sum(partitions) and
the divide.  All matmul operands are fp16 (1 col/cycle on the PE; more
precise than bf16 at these magnitudes), PSUM accumulation fp32; Q/K/V are
pre-cast to fp16 on the host, halving input DMA bytes.

PSUM budget/partition: 2 x [128,1024] f32 S^T staging (8KB) + 4 x [128,512]
f32 out accumulators (8KB) = 16KB exactly.

Host side only re-lays-out data: Q/K are fed pre-transposed [d, s], V as
[k_local, ktile, d]; the returned out^T [d, s] (unnormalized, fp16) is
divided by the denominators and transposed back.
"""

import numpy as np

import concourse.bass as bass
import concourse.mybir as mybir
import concourse.tile as tile
from concourse import bacc, bass_utils
from concourse.masks import make_identity, make_lower_triangular

S, B, HQ, HKV, D = 2048, 2, 32, 8, 128
G = HQ // HKV                      # 4 query heads per kv head
NCORES = 8
NPAIRS = B * HKV                   # 16 (batch, kv-head) pairs
PAIRS_PER_CORE = NPAIRS // NCORES  # 2
HEADS_PER_CORE = PAIRS_PER_CORE * G  # 8
SCALE = 1.0 / float(np.sqrt(D))
QC = 512                           # q-chunk (PSUM bank) width
NQC = S // QC                      # 4
KT = 128                           # k-tile (partition) width
NKT = S // KT                      # 16

F32 = mybir.dt.float32
F16 = mybir.dt.float16


def head_groups():
    """Chunk groups per head: (kti, chunks, off).

    For k-tile kti (k rows [w, w+128)), live q columns are [w, S).  Chunks of
    512 q-columns c0..3 (c0 = kti//4) are processed in pairs so consecutive
    matmuls share the stationary operand and exp covers 1024 columns.  off is
    the causal offset inside the first chunk of the FIRST group only.
    """
    groups = []
    for kti in range(NKT):
        w = KT * kti
        c0 = w // QC
        off = w - QC * c0
        cs = list(range(c0, NQC))
        first = True
        while cs:
            take = cs[:2]
            cs = cs[2:]
            groups.append((kti, tuple(take), off if first else 0))
            first = False
    return groups


GROUPS = head_groups()  # 24 groups


def emit_core_program(tc, qt, kt, v, ot, rd):
    """qt: [8, D, S] f16 DRAM (Q^T per head), kt: [2, D, S] f16, v: [2, 128,
    NKT*D] f16, ot: [8, D, S] f16 out (unnormalized out^T), rd: [8, 128, S]
    f16 out (per-k-row partial exp sums; host reduces partitions)."""
    from contextlib import ExitStack

    nc = tc.nc
    with ExitStack() as ctx:
        _emit(ctx, tc, nc, qt, kt, v, ot, rd)


def _emit(ctx, tc, nc, qt, kt, v, ot, rd):
    singles = ctx.enter_context(tc.tile_pool(name="singles", bufs=1))
    kv_pool = ctx.enter_context(tc.tile_pool(name="kv", bufs=2))
    q_pool = ctx.enter_context(tc.tile_pool(name="q", bufs=3))
    p_pool = ctx.enter_context(tc.tile_pool(name="p", bufs=8))
    r_pool = ctx.enter_context(tc.tile_pool(name="r", bufs=2))
    ob_pool = ctx.enter_context(tc.tile_pool(name="ob", bufs=4))
    ps_s = ctx.enter_context(tc.tile_pool(name="ps_s", bufs=2, space="PSUM"))
    ps_o = ctx.enter_context(tc.tile_pool(name="ps_o", bufs=4, space="PSUM"))

    # One-group-deep software pipeline over the whole program: the AV
    # matmuls for group i are emitted after the QK matmuls of group i+1, so
    # the PE always has runnable work while ACT exponentiates group i+1.
    pending = []  # [(head_ctx, kti, cs, off, p_tile)]

    def flush_pending():
        hctx, kti, cs, off, p_t = pending.pop(0)
        v_sb_, o_tiles_, head_ = hctx["v_sb"], hctx["o"], hctx["head"]
        for j, c in enumerate(cs):
            o_c = off if j == 0 else 0
            t0 = QC * j + o_c
            nc.tensor.matmul(
                out=o_tiles_[c][:, o_c:QC],
                lhsT=v_sb_[:, D * kti:D * (kti + 1)],
                rhs=p_t[:, t0:QC * (j + 1)],
                start=(kti == 0), stop=(kti == 4 * c + 3),
            )
            if kti == 4 * c + 3:
                # chunk finished accumulating: drain PSUM -> SBUF f16 -> DRAM
                osb = ob_pool.tile([128, QC], F16, tag="osb",
                                   name=f"osb_{head_}_{c}")
                nc.vector.tensor_copy(out=osb[:], in_=o_tiles_[c][:])
                nc.sync.dma_start(
                    out=ot[head_][:, QC * c:QC * (c + 1)], in_=osb[:])

    # Input loads (fp16, pre-cast on host) are prefetched one head ahead.
    pair_res = {}
    q_res = {}

    def load_pair(pr, splits=(S,)):
        kt_sb = kv_pool.tile([D, S], F16, tag="kt", name=f"kt_{pr}")
        lo = 0
        for hi in splits:
            nc.gpsimd.dma_start(out=kt_sb[:, lo:hi], in_=kt[pr][:, lo:hi])
            lo = hi
        v_sb = kv_pool.tile([128, NKT * D], F16, tag="v", name=f"v_{pr}")
        nc.gpsimd.dma_start(out=v_sb[:], in_=v[pr])
        pair_res[pr] = (kt_sb, v_sb)

    def load_q(h, splits=(S,)):
        q_sb = q_pool.tile([D, S], F16, tag="q", name=f"q_{h}")
        lo = 0
        for hi in splits:
            nc.sync.dma_start(out=q_sb[:, lo:hi], in_=qt[h][:, lo:hi])
            lo = hi
        q_res[h] = q_sb

    # Startup: issue the first loads before anything else queues on the
    # DMA-trigger engines, split into column chunks so the first QK group
    # only waits for kt cols [0,128) + q cols [0,512).
    load_pair(0, splits=(KT, QC, 2 * QC, S))
    load_q(0, splits=(QC, 2 * QC, S))

    # Causal mask is applied INSIDE the QK accumulation: one extra matmul
    # adds trim[k, q] = -BIG where q < k (strict lower triangle) to the
    # S^T diagonal block in PSUM, so exp produces exact zeros there and
    # nothing sits between exp and the AV matmuls.
    identf = singles.tile([128, 128], F32)
    make_identity(nc, identf[:])
    ident = singles.tile([128, 128], F16)
    nc.scalar.copy(out=ident[:], in_=identf[:])
    trimf = singles.tile([128, 128], F32)
    make_lower_triangular(nc, trimf[:], val=-30000.0, diag=False)
    trim = singles.tile([128, 128], F16)
    nc.scalar.copy(out=trim[:], in_=trimf[:])
    for head in range(HEADS_PER_CORE):
        pair = head // G
        if head + 1 < HEADS_PER_CORE:
            if (head + 1) // G != pair:
                load_pair((head + 1) // G)
            load_q(head + 1)
        if True:
            kt_sb, v_sb = pair_res[pair]
            q_sb = q_res.pop(head)
            r_sb = r_pool.tile([128, S], F16, tag="r", name=f"r_{head}")
            o_tiles = [ps_o.tile([128, QC], F32, tag="o", name=f"o_{head}_{c}")
                       for c in range(NQC)]
            hctx = {"v_sb": v_sb, "o": o_tiles, "head": head}

            for kti, cs, off in GROUPS:
                w = KT * kti
                ncols = QC * len(cs)
                s_t = ps_s.tile([128, 2 * QC], F32, tag="s",
                                name=f"s_{head}_{kti}_{cs[0]}")
                # QK^T: consecutive matmuls share lhsT (k-tile of K^T).
                # The -BIG causal-mask matmul OPENS the diagonal block's
                # accumulation chain (start=True) and the QK matmul over the
                # same exact region closes it, so the mask never sits last
                # before exp — exp's dependency is the final QK matmul.
                # The chain members stay contiguous on the PE (interleaving
                # another matmul inside an open group faults the exec unit).
                has_diag = QC * cs[0] <= w < QC * (cs[0] + 1)
                if has_diag:
                    nc.tensor.matmul(
                        out=s_t[:, off:off + KT],
                        lhsT=ident[:],
                        rhs=trim[:],
                        start=True, stop=False,
                    )
                    nc.tensor.matmul(
                        out=s_t[:, off:off + KT],
                        lhsT=kt_sb[:, w:w + KT],
                        rhs=q_sb[:, QC * cs[0] + off:QC * cs[0] + off + KT],
                        start=False, stop=True,
                    )
                    if off + KT < QC:
                        nc.tensor.matmul(
                            out=s_t[:, off + KT:QC],
                            lhsT=kt_sb[:, w:w + KT],
                            rhs=q_sb[:, QC * cs[0] + off + KT:QC * (cs[0] + 1)],
                            start=True, stop=True,
                        )
                    rest = list(range(1, len(cs)))
                else:
                    rest = list(range(len(cs)))
                for j in rest:
                    c = cs[j]
                    o_c = off if j == 0 else 0
                    nc.tensor.matmul(
                        out=s_t[:, QC * j + o_c:QC * (j + 1)],
                        lhsT=kt_sb[:, w:w + KT],
                        rhs=q_sb[:, QC * c + o_c:QC * (c + 1)],
                        start=True, stop=True,
                    )
                # exp on ACT (one instruction for the whole group)
                p_t = p_pool.tile([128, 2 * QC], F16, tag="p",
                                  name=f"p_{head}_{kti}_{cs[0]}")
                nc.scalar.activation(
                    p_t[:, off:ncols], s_t[:, off:ncols],
                    mybir.ActivationFunctionType.Exp, scale=SCALE)
                # denominator partials: R += P^T (Vector engine, fp16)
                rcol = slice(QC * cs[0] + off, QC * (cs[-1] + 1))
                if kti == 0:
                    nc.vector.tensor_copy(
                        out=r_sb[:, rcol], in_=p_t[:, off:ncols])
                else:
                    nc.vector.tensor_add(
                        r_sb[:, rcol], r_sb[:, rcol], p_t[:, off:ncols])

                if head == HEADS_PER_CORE - 1 and kti == 4 * cs[0] + 3:
                    # last head: stream R out chunk-by-chunk as each chunk's
                    # final add lands, so only 128KB remains in the tail
                    cc = cs[0]
                    nc.sync.dma_start(
                        out=rd[head][:, QC * cc:QC * (cc + 1)],
                        in_=r_sb[:, QC * cc:QC * (cc + 1)])

                pending.append((hctx, kti, cs, off, p_t))
                if len(pending) > 1:
                    flush_pending()

            if head < HEADS_PER_CORE - 1:
                # R is complete once the last group's add ran; DMA it out
                nc.sync.dma_start(out=rd[head], in_=r_sb[:])


    while pending:
        flush_pending()


_CACHED_NC = None


def build_program():
    global _CACHED_NC
    if _CACHED_NC is not None:
        return _CACHED_NC
    nc = bacc.Bacc("TRN2", target_bir_lowering=False, debug=False,
                   num_devices=NCORES)
    qt = nc.dram_tensor("qt", [HEADS_PER_CORE, D, S], F16,
                        kind="ExternalInput").ap()
    kt = nc.dram_tensor("kt", [PAIRS_PER_CORE, D, S], F16,
                        kind="ExternalInput").ap()
    v = nc.dram_tensor("v", [PAIRS_PER_CORE, 128, NKT * D], F16,
                       kind="ExternalInput").ap()
    ot = nc.dram_tensor("ot", [HEADS_PER_CORE, D, S], F16,
                        kind="ExternalOutput").ap()
    rd = nc.dram_tensor("rd", [HEADS_PER_CORE, 128, S], F16,
                        kind="ExternalOutput").ap()
    with tile.TileContext(nc) as tc:
        emit_core_program(tc, qt, kt, v, ot, rd)
    nc.compile()
    _CACHED_NC = nc
    return nc


def shard_inputs(query, key, value):
    """Full inputs -> list of 8 per-core in_maps (host-side relayout only)."""
    query = np.asarray(query, dtype=np.float32)
    key = np.asarray(key, dtype=np.float32)
    value = np.asarray(value, dtype=np.float32)

    # Q: [S,B,HQ,D] -> [B*HKV, G, D, S]
    qtall = np.ascontiguousarray(
        query.reshape(S, B, HKV, G, D).transpose(1, 2, 3, 4, 0),
        dtype=np.float16).reshape(NPAIRS, G, D, S)
    # K: [S,B,HKV,D] -> [B*HKV, D, S]
    ktall = np.ascontiguousarray(
        key.transpose(1, 2, 3, 0), dtype=np.float16).reshape(NPAIRS, D, S)
    # V: [S,B,HKV,D] -> [B*HKV, k_local=128, NKT*D]
    vall = np.ascontiguousarray(
        value.reshape(NKT, 128, B, HKV, D).transpose(2, 3, 1, 0, 4),
        dtype=np.float16).reshape(NPAIRS, 128, NKT * D)

    in_maps = []
    for c in range(NCORES):
        p0 = PAIRS_PER_CORE * c
        p1 = p0 + PAIRS_PER_CORE
        in_maps.append({
            "qt": np.ascontiguousarray(qtall[p0:p1].reshape(HEADS_PER_CORE, D, S)),
            "kt": np.ascontiguousarray(ktall[p0:p1]),
            "v": np.ascontiguousarray(vall[p0:p1]),
        })
    return in_maps


def unshard_output(results):
    """8 per-core {'ot','rd'} -> full [S, B, HQ, D] (normalize on host)."""
    ot = np.stack([np.asarray(r["ot"], dtype=np.float32) for r in results])
    rd = np.stack([np.asarray(r["rd"], dtype=np.float32) for r in results])
    denom = rd.sum(axis=2)                         # [8, 8, S]
    ot /= denom[:, :, None, :]                     # [8, 8, D, S]
    ot = ot.reshape(B, HKV, G, D, S)               # pairs major -> b, hkv
    out = np.ascontiguousarray(ot.transpose(4, 0, 1, 2, 3))  # [S,B,HKV,G,D]
    return out.reshape(S, B, HQ, D)


def kernel(query, key, value, _trace=False, _return_bkr=False):
    nc = build_program()
    in_maps = shard_inputs(query, key, value)
    bkr = bass_utils.run_bass_kernel_spmd(
        nc, in_maps, core_ids=list(range(NCORES)), trace=_trace)
    out = unshard_output(bkr.results)
    if _return_bkr:
        return out, bkr
    return out


if __name__ == "__main__":
    q = np.random.randn(S, B, HQ, D).astype(np.float32)
    k = np.random.randn(S, B, HKV, D).astype(np.float32)
    vv = np.random.randn(S, B, HKV, D).astype(np.float32)
    o = kernel(q, k, vv)
    print("out", o.shape, o.dtype, float(np.abs(o).max()))
